# revision 1
# baseline (speedup 1.0000x reference)
"""GIN message-passing encoder (3 layers) on 8 Trainium2 NeuronCores.

Problem: x_{l+1} = relu(BN(relu((x + agg(x)) @ W1 + b1) @ W2 + b2)),
agg[b, d] = sum over edges (s -> d) of x[b, s]; output = stack of the 3
layer outputs, shape [3, 16, 1024, 256].

Strategy
--------
- Data parallel over batch: B=16 split as 2 batch elements per core.
- The scatter-add is a dense matmul against a host-built (N x N) matrix
  Bm[s, d] = I[s, d] + multiplicity(edge s -> d), so
  m0 = (A + I) @ x = Bm^T-contracted matmul; the +x of GIN(eps=0) is the
  identity fold.
- Eval-mode BatchNorm is folded into W2/b2 on the host.
- All matmuls run as float32r (full PE rate at moving-free >= 256).
- Per layer per batch:
    step1: m0T[f, n]  = x-chunks^T (stationary) @ Bm-chunks (moving), PSUM
           -> DVE copy to SBUF (f32r)
    step2: h1T[g, n]  = W1 (stationary) @ m0T (moving)
           -> ACT Relu + per-partition bias b1 straight from PSUM (f32r)
    step3: y[n, gout] = h1T-chunks (stationary) @ W2' (moving)
           -> DVE add of broadcast bias b2', ACT Relu -> next x (f32r)
  x stays in "normal" layout [n, f] which is exactly what step1 consumes
  as stationary chunks, so no transposes anywhere.
"""

import os

import numpy as np

BN_EPS = 1e-5

B, N, F = 16, 1024, 256
L = 3
NCORES = 8
BPC = B // NCORES  # batch elements per core
P = 128
NT = N // P  # 8 node tiles
FT = F // P  # 2 feature tiles
HALF = 512   # moving free-dim chunk
NH = N // HALF  # 2 halves of the node dim

_cache: dict = {}


def _build_nc():
    import concourse.bacc as bacc
    import concourse.mybir as mybir
    import concourse.tile as tile

    F32 = mybir.dt.float32
    F32R = mybir.dt.float32r
    Relu = mybir.ActivationFunctionType.Relu
    Alu = mybir.AluOpType

    nc = bacc.Bacc()

    x0_d = nc.dram_tensor("x0", [BPC, N, F], F32R, kind="ExternalInput")
    bm_d = nc.dram_tensor("bm", [N, N], F32R, kind="ExternalInput")
    w1_d = nc.dram_tensor("w1", [L, F, F], F32R, kind="ExternalInput")
    w2_d = nc.dram_tensor("w2", [L, F, F], F32R, kind="ExternalInput")
    b1_d = nc.dram_tensor("b1", [P, L * FT], F32, kind="ExternalInput")
    b2_d = nc.dram_tensor("b2", [P, L, HALF], F32, kind="ExternalInput")
    out_d = nc.dram_tensor("out", [L, BPC, N, F], F32R, kind="ExternalOutput")

    with tile.TileContext(nc) as tc:
        with (
            tc.tile_pool(name="const", bufs=1) as cpool,
            tc.tile_pool(name="xp", bufs=2) as xpool,
            tc.tile_pool(name="work", bufs=3) as wpool,
            tc.tile_pool(name="yt", bufs=6) as ypool,
            tc.tile_pool(name="pm0", bufs=3, space="PSUM") as pm0,
            tc.tile_pool(name="ph1", bufs=2, space="PSUM") as ph1,
            tc.tile_pool(name="py", bufs=3, space="PSUM") as py,
        ):
            b_sb = cpool.tile([P, NT, N], F32R)
            w1_sb = cpool.tile([P, L, FT, F], F32R)
            w2_sb = cpool.tile([P, L, FT, F], F32R)
            b1_sb = cpool.tile([P, L * FT], F32)
            b2_sb = cpool.tile([P, L, HALF], F32)

            x_cur = xpool.tile([P, BPC, NT, F], F32R, tag="x")

            # Load order matters: per-DMA issue on the Sync queue is
            # ~620 ns, so coalesce chunks and stage the bytes the first
            # step-1 groups need (Bm half 0, batch 0) first.
            for k2 in range(0, NT, 2):
                nc.sync.dma_start(
                    b_sb[:, k2:k2 + 2, 0:HALF],
                    bm_d[k2 * P:(k2 + 2) * P, 0:HALF].rearrange(
                        "(c p) d -> p c d", p=P
                    ),
                )
            for k4 in range(0, NT, 4):
                nc.sync.dma_start(
                    x_cur[:, 0, k4:k4 + 4, :],
                    x0_d[0, k4 * P:(k4 + 4) * P, :].rearrange(
                        "(c p) f -> p c f", p=P
                    ),
                )
            nc.sync.dma_start(
                w1_sb[:, 0], w1_d[0].rearrange("(c p) g -> p c g", p=P)
            )
            nc.sync.dma_start(b1_sb[:], b1_d[:])
            nc.sync.dma_start(b2_sb[:], b2_d[:])
            for k2 in range(0, NT, 2):
                nc.sync.dma_start(
                    b_sb[:, k2:k2 + 2, HALF:N],
                    bm_d[k2 * P:(k2 + 2) * P, HALF:N].rearrange(
                        "(c p) d -> p c d", p=P
                    ),
                )
            for k4 in range(0, NT, 4):
                nc.sync.dma_start(
                    x_cur[:, 1, k4:k4 + 4, :],
                    x0_d[1, k4 * P:(k4 + 4) * P, :].rearrange(
                        "(c p) f -> p c f", p=P
                    ),
                )
            nc.sync.dma_start(
                w2_sb[:, 0], w2_d[0].rearrange("(c p) g -> p c g", p=P)
            )
            for l in range(1, L):
                nc.sync.dma_start(
                    w1_sb[:, l], w1_d[l].rearrange("(c p) g -> p c g", p=P)
                )
                nc.sync.dma_start(
                    w2_sb[:, l], w2_d[l].rearrange("(c p) g -> p c g", p=P)
                )

            for l in range(L):
                x_next = xpool.tile([P, BPC, NT, F], F32R, tag="x")
                for b in range(BPC):
                    # ---- step 1: m0T = (A + I) @ x, transposed layout ----
                    m0t = wpool.tile([P, FT, N], F32R, tag="m0t")
                    for half in range(NH):
                        for ft in range(FT):
                            ps = pm0.tile([P, HALF], F32, tag="pm0")
                            for k in range(NT):
                                nc.tensor.matmul(
                                    ps[:],
                                    x_cur[:, b, k, ft * P:(ft + 1) * P],
                                    b_sb[:, k, half * HALF:(half + 1) * HALF],
                                    start=(k == 0),
                                    stop=(k == NT - 1),
                                )
                            nc.vector.tensor_copy(
                                m0t[:, ft, half * HALF:(half + 1) * HALF], ps[:]
                            )
                    # ---- step 2: h1T = relu(W1^T-contract @ m0T + b1) ----
                    h1t = wpool.tile([P, FT, N], F32R, tag="h1t")
                    for gt in range(FT):
                        for half in range(NH):
                            ps = ph1.tile([P, HALF], F32, tag="ph1")
                            for fk in range(FT):
                                nc.tensor.matmul(
                                    ps[:],
                                    w1_sb[:, l, fk, gt * P:(gt + 1) * P],
                                    m0t[:, fk, half * HALF:(half + 1) * HALF],
                                    start=(fk == 0),
                                    stop=(fk == FT - 1),
                                )
                            nc.scalar.activation(
                                h1t[:, gt, half * HALF:(half + 1) * HALF],
                                ps[:],
                                Relu,
                                bias=b1_sb[:, l * FT + gt:l * FT + gt + 1],
                            )
                    # ---- step 3: y = h1 @ W2' + b2', relu -> next x ----
                    for tp in range(NT // 2):
                        ps = py.tile([P, 2, F], F32, tag="py")
                        for j in range(2):
                            nt = 2 * tp + j
                            for gk in range(FT):
                                nc.tensor.matmul(
                                    ps[:, j, :],
                                    h1t[:, gk, nt * P:(nt + 1) * P],
                                    w2_sb[:, l, gk, :],
                                    start=(gk == 0),
                                    stop=(gk == FT - 1),
                                )
                        ytmp = ypool.tile([P, 2, F], F32, tag="ytmp")
                        nc.vector.scalar_tensor_tensor(
                            ytmp[:],
                            ps[:],
                            1.0,
                            b2_sb[:, l, :].rearrange("p (a f) -> p a f", a=2),
                            op0=Alu.mult,
                            op1=Alu.add,
                        )
                        nc.scalar.activation(
                            x_next[:, b, 2 * tp:2 * tp + 2, :], ytmp[:], Relu
                        )
                        nc.sync.dma_start(
                            out_d[l, b, 2 * tp * P:(2 * tp + 2) * P, :].rearrange(
                                "(t p) f -> p t f", p=P
                            ),
                            x_next[:, b, 2 * tp:2 * tp + 2, :],
                        )
                x_cur = x_next

    nc.finalize()
    return nc


def kernel(h, edge_index, W1, b1, W2, b2, gamma, beta, run_mean, run_var):
    from concourse.bass_utils import run_bass_kernel_spmd

    h = np.asarray(h, dtype=np.float32)
    edge_index = np.asarray(edge_index)
    W1 = np.asarray(W1, dtype=np.float32)
    b1 = np.asarray(b1, dtype=np.float32)
    W2 = np.asarray(W2, dtype=np.float32)
    b2 = np.asarray(b2, dtype=np.float32)
    gamma = np.asarray(gamma, dtype=np.float32)
    beta = np.asarray(beta, dtype=np.float32)
    run_mean = np.asarray(run_mean, dtype=np.float32)
    run_var = np.asarray(run_var, dtype=np.float32)

    # host-side preprocessing
    src = edge_index[0].astype(np.int64)
    dst = edge_index[1].astype(np.int64)
    bm = np.zeros((N, N), dtype=np.float32)
    np.add.at(bm, (src, dst), 1.0)
    bm[np.arange(N), np.arange(N)] += 1.0

    inv = (gamma / np.sqrt(run_var + BN_EPS)).astype(np.float32)      # [L, F]
    w2f = (W2 * inv[:, None, :]).astype(np.float32)                   # [L, F, F]
    b2f = (b2 * inv + beta - run_mean * inv).astype(np.float32)       # [L, F]

    # b1 as per-partition scalars: [P, L*FT], column l*FT+gt = b1[l, gt*128:...]
    b1r = np.ascontiguousarray(
        b1.reshape(L, FT, P).transpose(2, 0, 1).reshape(P, L * FT)
    )
    # b2' broadcast across partitions, twice along free (for [P, 2, F] pairs)
    b2r = np.ascontiguousarray(
        np.broadcast_to(
            np.concatenate([b2f, b2f], axis=1)[None], (P, L, HALF)
        )
    )

    if "nc" not in _cache:
        _cache["nc"] = _build_nc()
    nc = _cache["nc"]

    in_maps = []
    for c in range(NCORES):
        in_maps.append({
            "x0": np.ascontiguousarray(h[c * BPC:(c + 1) * BPC]),
            "bm": bm,
            "w1": W1,
            "w2": w2f,
            "b1": b1r,
            "b2": b2r,
        })

    trace = os.environ.get("KERNEL_TRACE") == "1"
    res = run_bass_kernel_spmd(
        nc, in_maps, core_ids=list(range(NCORES)), trace=trace
    )
    _cache["last_results"] = res
    return np.concatenate([r["out"] for r in res.results], axis=1)



# revision 7
# speedup vs baseline: 1.0082x; 1.0082x over previous
"""GIN message-passing encoder (3 layers) on 8 Trainium2 NeuronCores.

Problem: x_{l+1} = relu(BN(relu((x + agg(x)) @ W1 + b1) @ W2 + b2)),
agg[b, d] = sum over edges (s -> d) of x[b, s]; output = stack of the 3
layer outputs, shape [3, 16, 1024, 256].

Strategy
--------
- Data parallel over batch: B=16 split as 2 batch elements per core.
- The scatter-add is a dense matmul against a host-built (N x N) matrix
  Bm[s, d] = I[s, d] + multiplicity(edge s -> d); the +x of GIN(eps=0)
  is the identity fold.
- Eval-mode BatchNorm is folded into W2/b2 on the host.
- step1 runs in fp8e4 DoubleRow mode (0.5 cycles/row, K=256/instr):
  Bm is exact in fp8 (small ints); x is split x = x_hi + x_lo with two
  e4m3 passes, residual error ~2^-8 relative. x0's split is host-side;
  x1/x2 are split on-device (ACT relu->fp8 + DVE subtract).
- b2 bias enters step3's PSUM via a ones-matmul (lhsT=ones[128,128],
  rhs has b2' on partition 0, zeros elsewhere), so ACT applies relu
  straight from PSUM; no DVE broadcast-add.
- MLP matmuls stay float32r (full PE rate at moving-free >= 256).
- Input DMAs are host-preswizzled (straight per-partition runs) and
  issued across sync/vector/scalar/gpsimd queues to cut the ~620ns
  per-DMA issue serialization; outputs stream on the sync queue.
"""

import os

import numpy as np

BN_EPS = 1e-5

B, N, F = 16, 1024, 256
L = 3
NCORES = 8
BPC = B // NCORES  # batch elements per core
P = 128
NT = N // P   # 8 node tiles
FT = F // P   # 2 feature tiles
KK = N // 256  # 4 double-chunks of the contraction dim (DoubleRow K=256)
HALF = 512    # moving free-dim chunk
NH = N // HALF  # 2 halves of the node dim

_cache: dict = {}


def _build_nc():
    import concourse.bacc as bacc
    import concourse.mybir as mybir
    import concourse.tile as tile

    F32 = mybir.dt.float32
    F32R = mybir.dt.float32r
    F8 = mybir.dt.float8e4
    Relu = mybir.ActivationFunctionType.Relu
    Alu = mybir.AluOpType
    DR = mybir.MatmulPerfMode.DoubleRow

    nc = bacc.Bacc()

    x0hi_d = nc.dram_tensor("x0hi", [BPC, P, KK, 2, F], F8, kind="ExternalInput")
    x0lo_d = nc.dram_tensor("x0lo", [BPC, P, KK, 2, F], F8, kind="ExternalInput")
    bm_d = nc.dram_tensor("bm", [P, KK, 2, N], F8, kind="ExternalInput")
    w1_d = nc.dram_tensor("w1", [P, L, FT, F], F32R, kind="ExternalInput")
    w2_d = nc.dram_tensor("w2", [P, L, FT, F], F32R, kind="ExternalInput")
    b1_d = nc.dram_tensor("b1", [P, L * FT], F32, kind="ExternalInput")
    b2_d = nc.dram_tensor("b2", [P, L, F], F32R, kind="ExternalInput")
    ones_d = nc.dram_tensor("ones", [P, P], F32R, kind="ExternalInput")
    out_d = nc.dram_tensor("out", [L, BPC, N, F], F32R, kind="ExternalOutput")

    with tile.TileContext(nc) as tc:
        with (
            tc.tile_pool(name="const", bufs=1) as cpool,
            tc.tile_pool(name="x8", bufs=2) as xpool,
            tc.tile_pool(name="m0", bufs=2) as wpool,
            tc.tile_pool(name="h1", bufs=2) as hpool,
            tc.tile_pool(name="yt", bufs=6) as ypool,
            tc.tile_pool(name="pm0", bufs=2, space="PSUM") as pm0,
            tc.tile_pool(name="ph1", bufs=2, space="PSUM") as ph1,
            tc.tile_pool(name="py", bufs=2, space="PSUM") as py,
        ):
            bm_sb = cpool.tile([P, KK, 2, N], F8)
            w1_sb = cpool.tile([P, L, FT, F], F32R)
            w2_sb = cpool.tile([P, L, FT, F], F32R)
            b1_sb = cpool.tile([P, L * FT], F32)
            b2z_sb = cpool.tile([P, L, F], F32R)
            ones_sb = cpool.tile([P, P], F32R)

            xhi = xpool.tile([P, BPC, KK, 2, F], F8, tag="xhi")
            xlo = xpool.tile([P, BPC, KK, 2, F], F8, tag="xlo")

            # Input DMAs spread across engine queues (issue is ~620ns each
            # on one queue); host pre-swizzles so every transfer is straight
            # contiguous per-partition runs.
            nc.sync.dma_start(bm_sb[:, 0:2], bm_d[:, 0:2])
            nc.sync.dma_start(bm_sb[:, 2:4], bm_d[:, 2:4])
            nc.scalar.dma_start(xhi[:, 0], x0hi_d[0])
            nc.scalar.dma_start(xlo[:, 0], x0lo_d[0])
            nc.gpsimd.dma_start(xhi[:, 1], x0hi_d[1])
            nc.gpsimd.dma_start(xlo[:, 1], x0lo_d[1])
            nc.scalar.dma_start(w1_sb[:], w1_d[:])
            nc.scalar.dma_start(w2_sb[:], w2_d[:])
            nc.scalar.dma_start(ones_sb[:], ones_d[:])
            nc.scalar.dma_start(b2z_sb[:], b2_d[:])
            nc.gpsimd.dma_start(b1_sb[:], b1_d[:])

            for l in range(L):
                last = l == L - 1
                if not last:
                    nxhi = xpool.tile([P, BPC, KK, 2, F], F8, tag="xhi")
                    nxlo = xpool.tile([P, BPC, KK, 2, F], F8, tag="xlo")
                for b in range(BPC):
                    # ---- step 1: m0T = (A + I) @ (x_hi + x_lo), fp8 DR ----
                    m0t = wpool.tile([P, FT, N], F32R, tag="m0t")
                    for ft in range(FT):
                        ps = pm0.tile([P, NH * HALF], F32, tag="pm0")
                        for half in range(NH):
                            idx = 0
                            for kk in range(KK):
                                for xx in (xhi, xlo):
                                    nc.tensor.matmul(
                                        ps[:, half * HALF:(half + 1) * HALF],
                                        xx[:, b, kk, :, ft * P:(ft + 1) * P],
                                        bm_sb[:, kk, :,
                                              half * HALF:(half + 1) * HALF],
                                        start=(idx == 0),
                                        stop=(idx == 2 * KK - 1),
                                        perf_mode=DR,
                                    )
                                    idx += 1
                        nc.vector.tensor_copy(m0t[:, ft, :], ps[:])
                    # ---- step 2: h1T = relu(W1^T-contract @ m0T + b1) ----
                    h1t = hpool.tile([P, FT, N], F32R, tag="h1t")
                    for gt in range(FT):
                        for half in range(NH):
                            ps2 = ph1.tile([P, HALF], F32, tag="ph1")
                            for fk in range(FT):
                                nc.tensor.matmul(
                                    ps2[:],
                                    w1_sb[:, l, fk, gt * P:(gt + 1) * P],
                                    m0t[:, fk, half * HALF:(half + 1) * HALF],
                                    start=(fk == 0),
                                    stop=(fk == FT - 1),
                                )
                            nc.scalar.activation(
                                h1t[:, gt, half * HALF:(half + 1) * HALF],
                                ps2[:],
                                Relu,
                                bias=b1_sb[:, l * FT + gt:l * FT + gt + 1],
                            )
                    # ---- step 3: y = relu(h1 @ W2' + b2') -> out + next x ----
                    for tp in range(NT // 2):
                        ps3 = py.tile([P, 2, F], F32, tag="py")
                        for j in range(2):
                            nt = 2 * tp + j
                            nc.tensor.matmul(
                                ps3[:, j, :], ones_sb[:], b2z_sb[:, l, :],
                                start=True, stop=False,
                            )
                            for gk in range(FT):
                                nc.tensor.matmul(
                                    ps3[:, j, :],
                                    h1t[:, gk, nt * P:(nt + 1) * P],
                                    w2_sb[:, l, gk, :],
                                    start=False,
                                    stop=(gk == FT - 1),
                                )
                        ynorm = ypool.tile([P, 2, F], F32R, tag="y")
                        nc.scalar.activation(ynorm[:], ps3[:], Relu)
                        nc.sync.dma_start(
                            out_d[l, b, 2 * tp * P:(2 * tp + 2) * P, :].rearrange(
                                "(t p) f -> p t f", p=P
                            ),
                            ynorm[:],
                        )
                        if not last:
                            nc.scalar.activation(
                                nxhi[:, b, tp, :, :], ps3[:], Relu
                            )
                            nc.vector.scalar_tensor_tensor(
                                nxlo[:, b, tp, :, :],
                                nxhi[:, b, tp, :, :],
                                -1.0,
                                ynorm[:],
                                op0=Alu.mult,
                                op1=Alu.add,
                            )
                if not last:
                    xhi, xlo = nxhi, nxlo

    nc.finalize()
    return nc


def kernel(h, edge_index, W1, b1, W2, b2, gamma, beta, run_mean, run_var):
    import ml_dtypes
    from concourse.bass_utils import run_bass_kernel_spmd

    f8 = ml_dtypes.float8_e4m3

    h = np.asarray(h, dtype=np.float32)
    edge_index = np.asarray(edge_index)
    W1 = np.asarray(W1, dtype=np.float32)
    b1 = np.asarray(b1, dtype=np.float32)
    W2 = np.asarray(W2, dtype=np.float32)
    b2 = np.asarray(b2, dtype=np.float32)
    gamma = np.asarray(gamma, dtype=np.float32)
    beta = np.asarray(beta, dtype=np.float32)
    run_mean = np.asarray(run_mean, dtype=np.float32)
    run_var = np.asarray(run_var, dtype=np.float32)

    # host-side preprocessing
    src = edge_index[0].astype(np.int64)
    dst = edge_index[1].astype(np.int64)
    bm = np.zeros((N, N), dtype=np.float32)
    np.add.at(bm, (src, dst), 1.0)
    bm[np.arange(N), np.arange(N)] += 1.0
    # fp8 exact for small integer counts; DoubleRow layout [P, KK, 2, N]
    bm8 = np.ascontiguousarray(
        bm.astype(f8).reshape(KK, 2, P, N).transpose(2, 0, 1, 3)
    )

    # x0 split into fp8 hi + lo on the host, swizzled to [B, P, KK, 2, F]
    xhi8 = h.astype(f8)
    xlo8 = (h - xhi8.astype(np.float32)).astype(f8)

    def swiz(a):
        return np.ascontiguousarray(
            a.reshape(B, KK, 2, P, F).transpose(0, 3, 1, 2, 4)
        )

    xhi8s, xlo8s = swiz(xhi8), swiz(xlo8)

    inv = (gamma / np.sqrt(run_var + BN_EPS)).astype(np.float32)      # [L, F]
    w2f = (W2 * inv[:, None, :]).astype(np.float32)                   # [L, F, F]
    b2f = (b2 * inv + beta - run_mean * inv).astype(np.float32)       # [L, F]

    # weights swizzled to [P, L, FT, F] (contraction chunk on partitions)
    w1s = np.ascontiguousarray(W1.reshape(L, FT, P, F).transpose(2, 0, 1, 3))
    w2s = np.ascontiguousarray(w2f.reshape(L, FT, P, F).transpose(2, 0, 1, 3))
    # b1 as per-partition scalars: [P, L*FT]
    b1r = np.ascontiguousarray(
        b1.reshape(L, FT, P).transpose(2, 0, 1).reshape(P, L * FT)
    )
    # b2' on partition 0 only; ones-matmul broadcasts it into step3's PSUM
    b2r = np.zeros((P, L, F), dtype=np.float32)
    b2r[0] = b2f
    ones_h = np.ones((P, P), dtype=np.float32)

    if "nc" not in _cache:
        _cache["nc"] = _build_nc()
    nc = _cache["nc"]

    in_maps = []
    for c in range(NCORES):
        in_maps.append({
            "x0hi": np.ascontiguousarray(xhi8s[c * BPC:(c + 1) * BPC]),
            "x0lo": np.ascontiguousarray(xlo8s[c * BPC:(c + 1) * BPC]),
            "bm": bm8,
            "w1": w1s,
            "w2": w2s,
            "b1": b1r,
            "b2": b2r,
            "ones": ones_h,
        })

    trace = os.environ.get("KERNEL_TRACE") == "1"
    res = run_bass_kernel_spmd(
        nc, in_maps, core_ids=list(range(NCORES)), trace=trace
    )
    _cache["last_results"] = res
    return np.concatenate([r["out"] for r in res.results], axis=1)


# revision 15
# speedup vs baseline: 1.2948x; 1.2842x over previous
"""GIN message-passing encoder (3 layers) on 8 Trainium2 NeuronCores.

Problem: x_{l+1} = relu(BN(relu((x + agg(x)) @ W1 + b1) @ W2 + b2)),
agg[b, d] = sum over edges (s -> d) of x[b, s]; output = stack of the 3
layer outputs, shape [3, 16, 1024, 256].

Strategy
--------
- Data parallel over batch: B=16 split as 2 batch elements per core.
- The scatter-add is a dense matmul against a host-built (N x N) matrix
  Bm[s, d] = I[s, d] + multiplicity(edge s -> d); the +x of GIN(eps=0)
  is the identity fold.
- Eval-mode BatchNorm is folded into W2/b2 on the host.
- step1 runs in fp8e4 DoubleRow mode (K=256 per instruction, halving
  the number of PSUM accumulation passes vs f32r's K=128): Bm is exact
  in fp8 (small ints); x is quantized e4m3 single-pass (measured rel
  err ~1.1e-2 vs the 2e-2 gate; hi/lo compensation would double the
  passes and erase the speedup). x0 is quantized host-side; x1/x2
  on-device (second ACT relu from step3's PSUM with fp8 output).
- b2 bias enters step3's PSUM via a ones-matmul (lhsT=ones[128,128],
  rhs has b2' on partition 0, zeros elsewhere), so ACT applies relu
  straight from PSUM; no DVE broadcast-add.
- MLP matmuls stay float32r (full PE rate at moving-free >= 256).
- Input DMAs are host-preswizzled (straight per-partition runs) and
  issued across sync/vector/scalar/gpsimd queues to cut the ~620ns
  per-DMA issue serialization; outputs stream on the sync queue.
"""

import os

import numpy as np

BN_EPS = 1e-5

B, N, F = 16, 1024, 256
L = 3
NCORES = 8
BPC = B // NCORES  # batch elements per core
P = 128
NT = N // P   # 8 node tiles
FT = F // P   # 2 feature tiles
KK = N // 256  # 4 double-chunks of the contraction dim (DoubleRow K=256)
HALF = 512    # moving free-dim chunk
NH = N // HALF  # 2 halves of the node dim

_cache: dict = {}


def _build_nc():
    import concourse.bacc as bacc
    import concourse.mybir as mybir
    import concourse.tile as tile

    F32 = mybir.dt.float32
    F32R = mybir.dt.float32r
    F8 = mybir.dt.float8e4
    Relu = mybir.ActivationFunctionType.Relu
    Alu = mybir.AluOpType
    DR = mybir.MatmulPerfMode.DoubleRow

    nc = bacc.Bacc()

    x0hi_d = nc.dram_tensor("x0hi", [BPC, P, KK, 2, F], F8, kind="ExternalInput")
    bm_d = nc.dram_tensor("bm", [P, KK, 2, N], F8, kind="ExternalInput")
    w1_d = nc.dram_tensor("w1", [P, L, FT, F], F32R, kind="ExternalInput")
    w2_d = nc.dram_tensor("w2", [P, L, FT, F], F32R, kind="ExternalInput")
    b1_d = nc.dram_tensor("b1", [P, L * FT], F32, kind="ExternalInput")
    b2_d = nc.dram_tensor("b2", [P, L, 2 * F], F32R, kind="ExternalInput")
    ones_d = nc.dram_tensor("ones", [P, P], F32R, kind="ExternalInput")
    out_d = nc.dram_tensor("out", [L, BPC, N, F], F32R, kind="ExternalOutput")

    with tile.TileContext(nc) as tc:
        with (
            tc.tile_pool(name="const", bufs=1) as cpool,
            tc.tile_pool(name="x8", bufs=2) as xpool,
            tc.tile_pool(name="m0", bufs=2) as wpool,
            tc.tile_pool(name="h1", bufs=2) as hpool,
            tc.tile_pool(name="yt", bufs=6) as ypool,
            tc.tile_pool(name="pm0", bufs=2, space="PSUM") as pm0,
            tc.tile_pool(name="ph1", bufs=2, space="PSUM") as ph1,
            tc.tile_pool(name="py", bufs=2, space="PSUM") as py,
        ):
            bm_sb = cpool.tile([P, KK, 2, N], F8)
            w1_sb = cpool.tile([P, L, FT, F], F32R)
            w2_sb = cpool.tile([P, L, FT, F], F32R)
            b1_sb = cpool.tile([P, L * FT], F32)
            b2z_sb = cpool.tile([P, L, 2 * F], F32R)
            ones_sb = cpool.tile([P, P], F32R)

            xhi = xpool.tile([P, BPC, KK, 2, F], F8, tag="xhi")

            # Input DMAs spread across engine queues (issue is ~620ns each
            # on one queue); host pre-swizzles so every transfer is straight
            # contiguous per-partition runs.
            nc.sync.dma_start(bm_sb[:, 0:2], bm_d[:, 0:2])
            nc.sync.dma_start(bm_sb[:, 2:4], bm_d[:, 2:4])
            nc.scalar.dma_start(xhi[:, 0], x0hi_d[0])
            nc.gpsimd.dma_start(xhi[:, 1], x0hi_d[1])
            nc.scalar.dma_start(w1_sb[:], w1_d[:])
            nc.scalar.dma_start(w2_sb[:], w2_d[:])
            nc.scalar.dma_start(ones_sb[:], ones_d[:])
            nc.scalar.dma_start(b2z_sb[:], b2_d[:])
            nc.gpsimd.dma_start(b1_sb[:], b1_d[:])

            for l in range(L):
                last = l == L - 1
                if not last:
                    nxhi = xpool.tile([P, BPC, KK, 2, F], F8, tag="xhi")
                for b in range(BPC):
                    # ---- step 1: m0T = (A + I) @ x_q, fp8 DoubleRow ----
                    # halves inner so the stationary x-chunk is reused for
                    # two consecutive matmuls (amortizes LdWeights).
                    m0t = wpool.tile([P, FT, N], F32R, tag="m0t")
                    for ft in range(FT):
                        ps = pm0.tile([P, NH * HALF], F32, tag="pm0")
                        for kk in range(KK):
                            for half in range(NH):
                                nc.tensor.matmul(
                                    ps[:, half * HALF:(half + 1) * HALF],
                                    xhi[:, b, kk, :, ft * P:(ft + 1) * P],
                                    bm_sb[:, kk, :,
                                          half * HALF:(half + 1) * HALF],
                                    start=(kk == 0),
                                    stop=(kk == KK - 1),
                                    perf_mode=DR,
                                )
                        nc.vector.tensor_copy(m0t[:, ft, :], ps[:])
                    # ---- step 2: h1T = relu(W1^T-contract @ m0T + b1) ----
                    h1t = hpool.tile([P, FT, N], F32R, tag="h1t")
                    for gt in range(FT):
                        for half in range(NH):
                            ps2 = ph1.tile([P, HALF], F32, tag="ph1")
                            for fk in range(FT):
                                nc.tensor.matmul(
                                    ps2[:],
                                    w1_sb[:, l, fk, gt * P:(gt + 1) * P],
                                    m0t[:, fk, half * HALF:(half + 1) * HALF],
                                    start=(fk == 0),
                                    stop=(fk == FT - 1),
                                )
                            nc.scalar.activation(
                                h1t[:, gt, half * HALF:(half + 1) * HALF],
                                ps2[:],
                                Relu,
                                bias=b1_sb[:, l * FT + gt:l * FT + gt + 1],
                            )
                    # ---- step 3: y = relu(h1 @ W2' + b2') -> out + next x ----
                    for tp in range(NT // 2):
                        ps3 = py.tile([P, 2, F], F32, tag="py")
                        # one 512-wide ones-matmul seeds b2' into both
                        # j-halves of the PSUM tile
                        nc.tensor.matmul(
                            ps3[:, :, :], ones_sb[:], b2z_sb[:, l, :],
                            start=True, stop=False, skip_group_check=True,
                        )
                        for j in range(2):
                            nt = 2 * tp + j
                            for gk in range(FT):
                                nc.tensor.matmul(
                                    ps3[:, j, :],
                                    h1t[:, gk, nt * P:(nt + 1) * P],
                                    w2_sb[:, l, gk, :],
                                    start=False,
                                    stop=(gk == FT - 1),
                                    skip_group_check=True,
                                )
                        ynorm = ypool.tile([P, 2, F], F32R, tag="y")
                        nc.scalar.activation(ynorm[:], ps3[:], Relu)
                        nc.sync.dma_start(
                            out_d[l, b, 2 * tp * P:(2 * tp + 2) * P, :].rearrange(
                                "(t p) f -> p t f", p=P
                            ),
                            ynorm[:],
                        )
                        if not last:
                            nc.scalar.activation(
                                nxhi[:, b, tp, :, :], ps3[:], Relu
                            )
                if not last:
                    xhi = nxhi

    nc.finalize()
    return nc


def kernel(h, edge_index, W1, b1, W2, b2, gamma, beta, run_mean, run_var):
    import ml_dtypes
    from concourse.bass_utils import run_bass_kernel_spmd

    f8 = ml_dtypes.float8_e4m3

    h = np.asarray(h, dtype=np.float32)
    edge_index = np.asarray(edge_index)
    W1 = np.asarray(W1, dtype=np.float32)
    b1 = np.asarray(b1, dtype=np.float32)
    W2 = np.asarray(W2, dtype=np.float32)
    b2 = np.asarray(b2, dtype=np.float32)
    gamma = np.asarray(gamma, dtype=np.float32)
    beta = np.asarray(beta, dtype=np.float32)
    run_mean = np.asarray(run_mean, dtype=np.float32)
    run_var = np.asarray(run_var, dtype=np.float32)

    # host-side preprocessing
    src = edge_index[0].astype(np.int64)
    dst = edge_index[1].astype(np.int64)
    bm = np.zeros((N, N), dtype=np.float32)
    np.add.at(bm, (src, dst), 1.0)
    bm[np.arange(N), np.arange(N)] += 1.0
    # fp8 exact for small integer counts; DoubleRow layout [P, KK, 2, N]
    bm8 = np.ascontiguousarray(
        bm.astype(f8).reshape(KK, 2, P, N).transpose(2, 0, 1, 3)
    )

    # x0 quantized to fp8 on the host, swizzled to [B, P, KK, 2, F]
    xhi8s = np.ascontiguousarray(
        h.astype(f8).reshape(B, KK, 2, P, F).transpose(0, 3, 1, 2, 4)
    )

    inv = (gamma / np.sqrt(run_var + BN_EPS)).astype(np.float32)      # [L, F]
    w2f = (W2 * inv[:, None, :]).astype(np.float32)                   # [L, F, F]
    b2f = (b2 * inv + beta - run_mean * inv).astype(np.float32)       # [L, F]

    # weights swizzled to [P, L, FT, F] (contraction chunk on partitions)
    w1s = np.ascontiguousarray(W1.reshape(L, FT, P, F).transpose(2, 0, 1, 3))
    w2s = np.ascontiguousarray(w2f.reshape(L, FT, P, F).transpose(2, 0, 1, 3))
    # b1 as per-partition scalars: [P, L*FT]
    b1r = np.ascontiguousarray(
        b1.reshape(L, FT, P).transpose(2, 0, 1).reshape(P, L * FT)
    )
    # b2' (duplicated pair) on partition 0 only; the 512-wide ones-matmul
    # broadcasts it into both halves of step3's PSUM tile
    b2r = np.zeros((P, L, 2 * F), dtype=np.float32)
    b2r[0] = np.concatenate([b2f, b2f], axis=1)
    ones_h = np.ones((P, P), dtype=np.float32)

    if "nc" not in _cache:
        _cache["nc"] = _build_nc()
    nc = _cache["nc"]

    in_maps = []
    for c in range(NCORES):
        in_maps.append({
            "x0hi": np.ascontiguousarray(xhi8s[c * BPC:(c + 1) * BPC]),
            "bm": bm8,
            "w1": w1s,
            "w2": w2s,
            "b1": b1r,
            "b2": b2r,
            "ones": ones_h,
        })

    trace = os.environ.get("KERNEL_TRACE") == "1"
    res = run_bass_kernel_spmd(
        nc, in_maps, core_ids=list(range(NCORES)), trace=trace
    )
    _cache["last_results"] = res
    return np.concatenate([r["out"] for r in res.results], axis=1)
